# revision 33
# baseline (speedup 1.0000x reference)
"""Trainium2 Bass kernel for the additive-attention module.

reference:
    hidden = concat([adj, static, dynamic, broadcast(dec)], axis=1)   # [B, 4H, N]
    Wh     = tanh(einsum('hk,bkn->bhn', W[0], hidden))                # [B, H, N]
    attns  = einsum('h,bhn->bn', v[0,0], Wh)[:, None, :]              # [B, 1, N]
    out    = softmax(attns, axis=2)

Strategy (data-parallel over batch, 8 NeuronCores, 32 batches/core):
  - Split W[0] [H, 4H] into 4 HxH blocks. The dec block contributes a
    per-(b,h) bias (constant over n): bias = dec @ W4.T, computed on host
    (tiny). The three big blocks run as float32r matmuls on the PE array,
    accumulated in PSUM: Wh_pre[b] = W1@adj[b] + W2@static[b] + W3@dyn[b].
  - tanh(x + bias) fused on ScalarE (per-partition AP bias).
  - v-dot via PE: for batch j, lhsT = [128, 32] window of a zero-padded v
    buffer with v in column j -> matmul accumulates v.tanh(.) into row j of
    a [32, 500] PSUM scores tile. 32 such matmuls share one accumulation
    group per N-tile, so scores land batch-major in partitions with no
    cross-partition copies.
  - softmax on [32, 1000]: VectorE max (negated), ScalarE exp with bias=-max
    and accum_out running sum, VectorE reciprocal + tensor_scalar multiply.
  - The three inputs are host-packed into one [H, B, 3, N] array so each
    per-core DMA group of G batches is a single 128-partition transfer with
    48 KiB contiguous per-partition chunks on one HWDGE ring (near-peak HBM
    bandwidth; a single clean stream per SDMA engine preserves row locality).
"""

import sys

if "/opt/trn_rl_repo" not in sys.path:
    sys.path.insert(0, "/opt/trn_rl_repo")

from contextlib import ExitStack

import numpy as np

import concourse.tile as tile
from concourse import bacc, mybir
from concourse.bass_utils import run_bass_kernel_spmd

N_CORES = 8
B, H, N = 256, 128, 1000
BPC = B // N_CORES  # 32 batches per core
G = 4               # batches per DMA group
NTS = 500           # free-dim tile size (PSUM bank limit: 512 fp32)
NT = N // NTS
F32 = mybir.dt.float32
F32R = mybir.dt.float32r
BF16 = mybir.dt.bfloat16

_NC_CACHE = {}


def _build():
    nc = bacc.Bacc("TRN2", target_bir_lowering=False, debug=False, num_devices=N_CORES)
    # wt [H, 3H] ++ vpad [H, 2*BPC-1] ++ bias [H, BPC], packed so the consts
    # land in ONE transfer ahead of the x stream (per-transfer queue startup
    # is ~0.5 us; three separate transfers would delay the first x group)
    CW = 3 * H + (2 * BPC - 1) + BPC
    x = nc.dram_tensor("x", [H, BPC, 3, N], BF16, kind="ExternalInput").ap()
    cst = nc.dram_tensor("cst", [H, CW], BF16, kind="ExternalInput").ap()
    out = nc.dram_tensor("out", [BPC, N], F32, kind="ExternalOutput").ap()

    with tile.TileContext(nc) as tc, ExitStack() as ctx:
        consts = ctx.enter_context(tc.tile_pool(name="consts", bufs=1))
        inp = ctx.enter_context(tc.tile_pool(name="inp", bufs=8))
        acts = ctx.enter_context(tc.tile_pool(name="acts", bufs=8))
        pwh = ctx.enter_context(tc.tile_pool(name="pwh", bufs=5, space="PSUM"))
        psc = ctx.enter_context(tc.tile_pool(name="psc", bufs=1, space="PSUM"))
        warm = ctx.enter_context(tc.tile_pool(name="warm", bufs=1, space="PSUM"))
        smax = ctx.enter_context(tc.tile_pool(name="smax", bufs=1))

        cst_sb = consts.tile([H, CW], BF16, tag="cst")
        VOFF = 3 * H               # vpad column offset within cst
        BOFF = 3 * H + 2 * BPC - 1  # bias column offset within cst

        def load_consts():
            # on the SAME queue as the x stream, issued first: queue FIFO makes
            # it complete before the bulk transfers.  On any other queue the
            # tiny packets get round-robined 1:1 against 12 KiB x packets and
            # the bias lands ~10 us late, stalling the first tanh.
            nc.sync.dma_start(out=cst_sb[:], in_=cst[:])

        sc = [psc.tile([BPC, NTS], F32, tag=f"sc{t}", name=f"sc{t}") for t in range(NT)]

        # PE warm-up: HAM clock-gates the PE to K=4/8 (half rate) until it
        # sees ~2 windows (3.4 us each) of sustained matmul activity. The PE
        # is idle through the preamble + first DMA anyway, so burn that time
        # on dummy matmuls into a scratch PSUM bank that is never read; real
        # matmuls then start at full rate.
        NWARM = 10
        dum = consts.tile([H, 512], BF16, tag="dum")
        nc.gpsimd.memset(dum[:], 0.0)
        wp = warm.tile([H, 512], F32, tag="wp")
        for i in range(NWARM):
            nc.tensor.matmul(
                wp[:],
                lhsT=dum[:, :H],
                rhs=dum[:],
                start=(i == 0),
                stop=(i == NWARM - 1),
                skip_group_check=True,
            )

        def vdot(b, t, th):
            # accumulate v . tanh(Wh[b]) into row b of sc[t]
            nc.tensor.matmul(
                sc[t][:],
                lhsT=cst_sb[:, VOFF + BPC - 1 - b : VOFF + 2 * BPC - 1 - b],
                rhs=th[:],
                start=(b == 0),
                stop=(b == BPC - 1),
                skip_group_check=True,
            )

        # Small groups at BOTH ends: leading 1-batch transfers get the PE onto
        # real work ASAP (keeps HAM warm after the warm-up); trailing 1-batch
        # transfers bound the post-DMA compute tail. 2-batch steady state
        # keeps the per-group PE wait (~1.5 us) under the ~3 us HAM
        # re-throttle threshold that 4-batch groups would hit.
        sizes = [1, 1, 1, 1] + [2] * 13 + [1, 1]
        assert sum(sizes) == BPC
        # four-deep software pipeline: vdot(b,t) issues 12 main matmuls (~2.5 us)
        # after its tanh was issued, so the PE never stalls on scalar latency
        PDEPTH = 4
        pending = []
        b0 = 0
        for g, sz in enumerate(sizes):
            # one packed transfer per group: 12 KiB contiguous per partition
            xt = inp.tile([H, sz, 3, N], BF16, tag="x", name=f"xt{g}")
            if g == 0:
                # consts BEFORE the x stream: they share the same 16 DMA-engine
                # rings, and once the x stream saturates them a ~16 KiB bias
                # transfer can be starved for ~10 us, stalling the first tanh
                load_consts()
            nc.sync.dma_start(out=xt[:], in_=x[:, b0 : b0 + sz, :, :])
            for j in range(sz):
                b = b0 + j
                for t in range(NT):
                    s0 = t * NTS
                    pw = pwh.tile([H, NTS], F32, tag="pw")
                    for ti in range(3):
                        nc.tensor.matmul(
                            pw[:],
                            lhsT=cst_sb[:, ti * H : (ti + 1) * H],
                            rhs=xt[:, j, ti, s0 : s0 + NTS],
                            start=(ti == 0),
                            stop=(ti == 2),
                            skip_group_check=True,
                        )
                    th = acts.tile([H, NTS], BF16, tag="th")
                    nc.scalar.activation(
                        th[:],
                        pw[:],
                        mybir.ActivationFunctionType.Tanh,
                        bias=cst_sb[:, BOFF + b : BOFF + b + 1],
                    )
                    if len(pending) >= PDEPTH:
                        vdot(*pending.pop(0))
                    pending.append((b, t, th))
            b0 += sz
        for p in pending:
            vdot(*p)

        # softmax straight from the PSUM score tiles (no staging copies);
        # per-tile maxes so the t=0 max overlaps the final t=1 v-dot.
        mx = [smax.tile([BPC, 1], F32, tag=f"mx{t}", name=f"mx{t}") for t in range(NT)]
        for t in range(NT):
            nc.vector.tensor_reduce(
                out=mx[t][:], in_=sc[t][:], axis=mybir.AxisListType.X,
                op=mybir.AluOpType.max, negate=True,
            )
        negmax = smax.tile([BPC, 1], F32, tag="negmax")
        nc.vector.tensor_tensor(
            out=negmax[:], in0=mx[0][:], in1=mx[1][:], op=mybir.AluOpType.min
        )
        esb = smax.tile([BPC, N], F32, tag="esb")
        sums = [smax.tile([BPC, 1], F32, tag=f"sums{t}", name=f"sums{t}") for t in range(NT)]
        for t in range(NT):
            nc.scalar.activation(
                esb[:, t * NTS : (t + 1) * NTS], sc[t][:],
                mybir.ActivationFunctionType.Exp,
                bias=negmax[:], accum_out=sums[t][:],
            )
        rcp = smax.tile([BPC, 1], F32, tag="rcp")
        nc.vector.tensor_tensor(
            out=rcp[:], in0=sums[0][:], in1=sums[1][:], op=mybir.AluOpType.add
        )
        nc.vector.reciprocal(rcp[:], rcp[:])
        # scale and store per half so the first store overlaps the second scale
        for t in range(NT):
            cols = slice(t * NTS, (t + 1) * NTS)
            nc.vector.tensor_scalar_mul(esb[:, cols], esb[:, cols], rcp[:])
            nc.sync.dma_start(out=out[:, cols], in_=esb[:, cols])

    nc.compile()
    return nc


def _get_nc():
    if "nc" not in _NC_CACHE:
        _NC_CACHE["nc"] = _build()
    return _NC_CACHE["nc"]


def _prep_in_maps(adj_hidden, static_hidden, dynamic_hidden, decoder_hidden, v, W):
    import ml_dtypes

    bf16 = ml_dtypes.bfloat16
    f32 = lambda x: np.asarray(x, dtype=np.float32)
    # pack the three [B, H, N] tensors as [H, B, 3, N] in bf16: halves HBM
    # traffic (the kernel is DMA-bound) and per-core DMA groups are a single
    # transfer with 24 KiB contiguous per-partition chunks
    x_all = np.ascontiguousarray(
        np.stack(
            [
                f32(adj_hidden).astype(bf16),
                f32(static_hidden).astype(bf16),
                f32(dynamic_hidden).astype(bf16),
            ],
            axis=2,
        ).transpose(1, 0, 2, 3)
    )  # [H, B, 3, N]
    W0 = f32(W)[0]  # [H, 4H]
    # wt[k, i*H + h] = W0[h, i*H + k] : block i is the lhsT of W-block i
    wt_host = W0[:, : 3 * H].reshape(H, 3, H).transpose(2, 1, 0).reshape(H, 3 * H)
    vv = f32(v).reshape(H)
    vpad_host = np.zeros((H, 2 * BPC - 1), np.float32)
    vpad_host[:, BPC - 1] = vv
    dec = f32(decoder_hidden)  # [B, H]
    bias_all = dec @ W0[:, 3 * H :].T  # [B, H]

    in_maps = []
    for c in range(N_CORES):
        lo, hi = c * BPC, (c + 1) * BPC
        cst_host = np.concatenate(
            [wt_host, vpad_host, bias_all[lo:hi, :].T], axis=1
        ).astype(bf16)
        in_maps.append(
            {
                "x": np.ascontiguousarray(x_all[:, lo:hi, :, :]),
                "cst": cst_host,
            }
        )
    return in_maps


def _run(in_maps, trace=False, **kw):
    nc = _get_nc()
    res = run_bass_kernel_spmd(nc, in_maps, core_ids=list(range(N_CORES)), trace=trace, **kw)
    full = np.concatenate(
        [res.results[c]["out"][:, None, :] for c in range(N_CORES)], axis=0
    )
    return full, res


def kernel(adj_hidden, static_hidden, dynamic_hidden, decoder_hidden, v, W):
    in_maps = _prep_in_maps(adj_hidden, static_hidden, dynamic_hidden, decoder_hidden, v, W)
    full, _ = _run(in_maps, trace=False)
    return full

